# revision 54
# baseline (speedup 1.0000x reference)
"""MultiHeadAttn (post-LN, key-padding mask) Trainium2 Bass kernel, 8 cores.

Problem: h [S=2048, B=4, D=1024] f32; 16 heads x 64; key-padding mask [S, B];
out = LayerNorm(h + MHA(h)).

Sharding: core c handles batch b = c//2 and query half qh = c%2 (1024 query
rows), with all 16 heads and the full 2048-key context for that batch.
KV projections are recomputed by both cores of a batch pair (no collectives).

Per-core device pipeline (all matmuls bf16, fp32 accumulation in PSUM):
  - K^T/Q^T proj: stationary Wk/Wq column tiles, moving h^T -> [e, t] layout.
  - V proj: stationary h^T tiles, moving Wv -> natural [t, e] layout, stored
    with an appended ones column per head (gives softmax denominators via PV).
  - Attention per head pair: scores^T [j,i] via row-paired matmuls (two heads
    in row strips 0-63 / 64-127 of the PE array), exp via ScalarE with the
    key-padding bias as a per-partition bias and 1/sqrt(dh) as the scale,
    PV with ones-augmented V (M=65; row 64 accumulates the denominator),
    then normalize via reciprocal + partition broadcast + multiply.
  - Output proj: stationary attn_vec^T tiles, moving Wo; residual add + LN
    (bn_stats/bn_aggr) fused on DVE/ScalarE.
Next head pair's K/Q projections are interleaved into the attention loop
(borrowing scores-pool PSUM slots) so the PE stays busy under the ACT-bound
softmax stream.
"""
import numpy as np
import ml_dtypes

# ---- custom DVE exp: e^y = (p(y/32))^32 --------------------------------
# op1 (5 ALU stages): m = y*C0; p = (1+m) + m^2*(C1 + C2*m)  [f32]
# op2 (6 ALU stages): out = (in^32) * C0, C0 = per-partition 0/1 key mask
# Offloads part of the softmax exp stream from the ACT engine (the
# attention-loop bottleneck) to the otherwise idle DVE ALU pipeline.
# Fitted constants give <=0.55% rel err over |y|<=12 (f32), ~0.93% incl
# the bf16 store - the key-padding mask is exact (multiply by 0).
EXP_A = 0.5045829621045046
EXP_B = 0.16659614969050196


def _register_dve_exp():
    import concourse.dve_ops as dve_ops
    from concourse.dve_ops import DveOp
    from concourse.dve_spec import Spec, Src0, C0, C1, C2, One, sq, lower, _has_src1
    from concourse.dve_uop import DveOpSpec

    def reg(name, spec):
        if name in dve_ops._SUB_OPCODE_FOR_NAME:
            return next(o for o in dve_ops.OPS if o.name == name)
        row = max(dve_ops._SUB_OPCODE_FOR_NAME.values()) + 1
        shas = {}
        for ver in ("v3", "v4"):
            s = DveOpSpec(name=name, opcode=row, uops=lower(spec, ver=ver),
                          rd1_en=_has_src1(spec))
            shas[ver] = s.sha(ver)
        op = DveOp(name, spec, subdim=False, uops_sha=shas)
        dve_ops.OPS.append(op)
        dve_ops._SUB_OPCODE_FOR_NAME[name] = row
        dve_ops.CUSTOM_DVE_SPECS[name] = spec
        return op

    _m = Src0 * C0
    exp_poly = Spec(
        body=(One + _m) + sq(_m) * (C1 + C2 * _m),
        reference=lambda in0, in1, s0, s1, imm2: (
            (1.0 + in0 * s0) + (in0 * s0) ** 2 * (s1 + imm2 * (in0 * s0))
        ).astype(np.float32),
    )
    sq32_mask = Spec(
        body=sq(sq(sq(sq(sq(Src0))))) * C0,
        reference=lambda in0, in1, s0, s1, imm2: (
            (in0.astype(np.float64) ** 32) * s0
        ).astype(np.float32),
    )
    return reg("ANT_EXP32_POLY", exp_poly), reg("ANT_SQ32_MASK", sq32_mask)


N_HEAD, D_MODEL, D_HEAD = 16, 1024, 64
SEQ, BSZ = 2048, 4
QLEN = SEQ // 2
SCALE = 1.0 / D_HEAD ** 0.5
LN_EPS = 1e-5
P = 128
NSL = 512                   # matmul moving-operand slab (one PSUM bank fp32)
CT = D_MODEL // P           # 8 contraction tiles
ET = D_MODEL // P           # 8 e-tiles (2 heads each)
JT = SEQ // P               # 16 key tiles
JS = SEQ // NSL             # 4 key slabs
IS = QLEN // NSL            # 2 query slabs
TQ = QLEN // P              # 8 query-row tiles
HP = N_HEAD // 2            # 8 head pairs

_CACHE = {}


def _build():
    from contextlib import ExitStack
    import concourse.bass as bass
    import concourse.mybir as mybir
    import concourse.tile as tile
    from concourse import bacc

    dt = mybir.dt
    f32, bf16 = dt.float32, dt.bfloat16
    AF = mybir.ActivationFunctionType
    ALU = mybir.AluOpType
    EXP32_POLY, SQ32_MASK = _register_dve_exp()

    nc = bacc.Bacc(None, target_bir_lowering=False)

    f8 = dt.float8e4
    # all four projections run fp8e4m3 DoubleRow (two 128-contraction tiles
    # per PE pass): h is quantized as-is (|h|<~6), weights are pre-scaled
    # x32 on the host so their ~N(0,1/32) entries use the e4m3 normal range;
    # the 1/32 rides existing evacuation multiplies for free.
    hT = nc.dram_tensor("hT", [D_MODEL, SEQ], f8, kind="ExternalInput")
    hq = nc.dram_tensor("hq", [QLEN, D_MODEL], f32, kind="ExternalInput")
    wq = nc.dram_tensor("wq", [D_MODEL, D_MODEL], f8, kind="ExternalInput")
    wk = nc.dram_tensor("wk", [D_MODEL, D_MODEL], f8, kind="ExternalInput")
    wv = nc.dram_tensor("wv", [D_MODEL, D_MODEL], f8, kind="ExternalInput")
    wo = nc.dram_tensor("wo", [D_MODEL, D_MODEL], f8, kind="ExternalInput")
    mb = nc.dram_tensor("mb", [SEQ], f32, kind="ExternalInput")
    mm = nc.dram_tensor("mm", [SEQ], f32, kind="ExternalInput")
    gam = nc.dram_tensor("gam", [D_MODEL], f32, kind="ExternalInput")
    bet = nc.dram_tensor("bet", [D_MODEL], f32, kind="ExternalInput")
    out = nc.dram_tensor("out", [QLEN, D_MODEL], f32, kind="ExternalOutput")

    with tile.TileContext(nc) as tc, ExitStack() as ctx:
        persist = ctx.enter_context(tc.tile_pool(name="persist", bufs=1))

        # normalized attn vectors stored fp8 (x8 for e4m3 range), paired
        # along the contraction dim for the DoubleRow output projection
        avt_sb = [persist.tile([P, 2, QLEN], f8, name=f"avt{ep}")
                  for ep in range(ET // 2)]
        mask_sb = persist.tile([P, JT], f32, name="mask")
        mmul_sb = persist.tile([P, JT], f32, name="mmul")
        eps_sb = persist.tile([P, 1], f32, name="eps")

        nc.vector.memset(eps_sb, LN_EPS)

        nc.gpsimd.dma_start(out=mask_sb,
                            in_=bass.AP(tensor=mb, offset=0, ap=[[1, P], [P, JT]]))
        nc.gpsimd.dma_start(out=mmul_sb,
                            in_=bass.AP(tensor=mm, offset=0, ap=[[1, P], [P, JT]]))

        # ---- phase-3 weights: load early into the region wvp freed ---------
        w3p = ctx.enter_context(tc.tile_pool(name="w3p", bufs=1))
        wo_sb = [w3p.tile([P, 2, D_MODEL], f8, name=f"wo{cp}")
                 for cp in range(CT // 2)]
        gam_sb = w3p.tile([P, D_MODEL], f32, name="gamr")
        bet_sb = w3p.tile([P, D_MODEL], f32, name="betr")

        # ---- phase 1 scope: h^T residency + streamed W columns --------------
        # h^T lives as 4 contraction-pair tiles [128, 2, SEQ] fp8 so a
        # DoubleRow matmul consumes two 128-row tiles per pass.
        ph1_ctx = ExitStack()
        ph1 = ph1_ctx.enter_context(tc.tile_pool(name="ph1", bufs=1))
        ht_sb = [ph1.tile([P, 2, SEQ], f8, name=f"ht{cp}")
                 for cp in range(CT // 2)]
        # attention-lifetime tensors live in the phase-1 scope (not persist)
        # so the post phase gets their SBUF back for deep LN pipelining.
        # K^T/Q^T stay bf16 (softmax path); V and the probabilities pair key
        # tiles in fp8e5m2 for a DoubleRow PV (256-key contraction per pass).
        f8e5 = dt.float8e5
        kt_sb = [ph1.tile([P, SEQ], bf16, name=f"kt{e}") for e in range(ET)]
        qt_sb = [ph1.tile([P, QLEN], bf16, name=f"qt{e}") for e in range(ET)]
        # V pair-tiles carry 65 columns per head: 64 value dims plus an
        # all-ones column, so a single M=65 DoubleRow matmul accumulates
        # the attention vector AND its softmax denominator (row 64) - the
        # separate denominator matmuls disappear entirely.
        VW = D_HEAD + 1
        vp_sb = [ph1.tile([P, 2, N_HEAD, VW], f8e5, name=f"vp{jp}")
                 for jp in range(JT // 2)]
        for jp in range(JT // 2):
            nc.vector.memset(vp_sb[jp][:, :, :, D_HEAD:VW], 1.0)

        wcol = ph1_ctx.enter_context(tc.tile_pool(name="wcol", bufs=3))

        def load_wcol(w, e, tag):
            wc = wcol.tile([P, CT, P], f8, tag=tag, name=f"{tag}{e}")
            nc.sync.dma_start(
                out=wc,
                in_=w[:, e * P:(e + 1) * P].rearrange("(ct p) e -> p ct e", p=P))
            return wc

        DR = mybir.MatmulPerfMode.DoubleRow

        wc0 = load_wcol(wk, 0, "wkc")
        # stream h^T by key slab so the first K-proj group (slab 0) starts
        # after ~0.5 MB of DMA instead of the full 2 MB.
        for sl in range(JS):
            for c in range(CT):
                eng = nc.sync if (sl * CT + c) % 2 == 0 else nc.scalar
                eng.dma_start(
                    out=ht_sb[c // 2][:, c % 2, sl * NSL:(sl + 1) * NSL],
                    in_=hT[c * P:(c + 1) * P, sl * NSL:(sl + 1) * NSL])

        def kq_group(ps_ap, wc, moving, sl):
            """4 DoubleRow matmuls: one K/Q-proj output group into psum."""
            for cp in range(CT // 2):
                nc.tensor.matmul(ps_ap, wc[:, 2 * cp:2 * cp + 2, :],
                                 moving[cp][:, :, sl * NSL:(sl + 1) * NSL],
                                 start=(cp == 0), stop=(cp == CT // 2 - 1),
                                 perf_mode=DR)

        # prephase: K(0), Q(0), V (own pools, closed before attention).
        # Evacuations fold the 1/32 fp8 weight-scale compensation into the
        # psum->bf16 cast.
        W_INV = 1.0 / 32.0
        with tc.tile_pool(name="wvp", bufs=1) as wvp, \
             tc.tile_pool(name="psA", bufs=6, space="PSUM") as psA:
            wv_sb = [wvp.tile([P, 2, D_MODEL], f8, name=f"wv{cp}")
                     for cp in range(CT // 2)]
            for c in range(CT):
                nc.scalar.dma_start(out=wv_sb[c // 2][:, c % 2, :],
                                    in_=wv[c * P:(c + 1) * P, :])
            wc = wc0
            for j in range(JS):
                ps = psA.tile([P, NSL], f32, tag="psa", name=f"psk0_{j}")
                kq_group(ps, wc, ht_sb, j)
                nc.vector.tensor_scalar_mul(
                    kt_sb[0][:, j * NSL:(j + 1) * NSL], ps, W_INV)
            wc = load_wcol(wq, 0, "wqc")
            for i in range(IS):
                ps = psA.tile([P, NSL], f32, tag="psa", name=f"psq0_{i}")
                kq_group(ps, wc, ht_sb, i)
                nc.vector.tensor_scalar_mul(
                    qt_sb[0][:, i * NSL:(i + 1) * NSL], ps, W_INV)
            # V projection: stationary h^T pair tiles, moving Wv slabs
            for t in range(JT):
                for es in range(2):
                    ps = psA.tile([P, NSL], f32, tag="psa", name=f"psv{t}_{es}")
                    for cp in range(CT // 2):
                        nc.tensor.matmul(
                            ps, ht_sb[cp][:, :, t * P:(t + 1) * P],
                            wv_sb[cp][:, :, es * NSL:(es + 1) * NSL],
                            start=(cp == 0), stop=(cp == CT // 2 - 1),
                            perf_mode=DR)
                    nc.vector.tensor_scalar_mul(
                        vp_sb[t // 2][:, t % 2, es * 8:(es + 1) * 8, 0:D_HEAD],
                        ps[:, :].rearrange("p (h d) -> p h d", d=D_HEAD),
                        W_INV)

        def emit_pv(nc, av, hp, jp, pts):
            # DoubleRow PV: one M=65 pass per (head, slab) contracts a
            # 256-key pair-tile and accumulates av rows 0-63 + den row 64.
            # pts[i] is [128 keys, 2 key-tiles, 1024 = head-A | head-B slab]
            first, last = (jp == 0), (jp == JT // 2 - 1)
            for i in range(IS):
                for hb in range(2):
                    nc.tensor.matmul(
                        av[hb][i], vp_sb[jp][:, :, hp * 2 + hb, :],
                        pts[i][:, :, hb * NSL:(hb + 1) * NSL],
                        start=first, stop=last, perf_mode=DR)

        for c in range(CT):
            nc.scalar.dma_start(out=wo_sb[c // 2][:, c % 2, :],
                                in_=wo[c * P:(c + 1) * P, :])
        nc.gpsimd.dma_start(out=gam_sb,
                            in_=bass.AP(tensor=gam, offset=0, ap=[[0, P], [1, D_MODEL]]))
        nc.gpsimd.dma_start(out=bet_sb,
                            in_=bass.AP(tensor=bet, offset=0, ap=[[0, P], [1, D_MODEL]]))

        # ---- attention ------------------------------------------------------
        attn_ctx = ExitStack()
        scp = attn_ctx.enter_context(tc.tile_pool(name="scp", bufs=2, space="PSUM"))
        avp = attn_ctx.enter_context(tc.tile_pool(name="avp", bufs=4, space="PSUM"))
        ptp = attn_ctx.enter_context(tc.tile_pool(name="ptp", bufs=8))
        nrm = attn_ctx.enter_context(tc.tile_pool(name="nrm", bufs=3))
        exs = attn_ctx.enter_context(tc.tile_pool(name="exs", bufs=2))

        def emit_norm(hp, av):
            # den sits in row 64 of each av bank: copy it out, fast-approx
            # reciprocal, replicate across 64 partitions on the idle gpsimd
            # engine, then one STT per (head, slab) fusing the x8 fp8-range
            # scale and the multiply with the fp8 store.
            for i in range(IS):
                for hb in range(2):
                    d1 = nrm.tile([1, NSL], f32, tag="d1",
                                  name=f"d1_{hp}_{i}_{hb}")
                    nc.vector.tensor_copy(d1, av[hb][i][64:65, :])
                    r1 = nrm.tile([1, NSL], f32, tag="r1",
                                  name=f"r1_{hp}_{i}_{hb}")
                    nc.vector.reciprocal_approx_fast(r1, d1)
                    rep = nrm.tile([64, NSL], f32, tag="rep",
                                   name=f"rep{hp}_{i}_{hb}")
                    nc.gpsimd.partition_broadcast(rep, r1, channels=64)
                    nc.vector.scalar_tensor_tensor(
                        out=avt_sb[hp // 2][hb * 64:(hb + 1) * 64, hp % 2,
                                            i * NSL:(i + 1) * NSL],
                        in0=av[hb][i][0:64, :], scalar=8.0,
                        in1=rep, op0=ALU.mult, op1=ALU.mult)

        # PV (and the hp-final normalization) trail the scores/exp stream by
        # two key tiles GLOBALLY - the pipeline flows across head-pair
        # boundaries, so the last exps of one hp overlap the first scores
        # of the next instead of draining into a bubble.
        avs = {}
        pv_pending = []

        def flush_pv(upto):
            while len(pv_pending) > upto:
                php, pjp, ppts = pv_pending.pop(0)
                emit_pv(nc, avs[php], php, pjp, ppts)
                if pjp == JT // 2 - 1:
                    emit_norm(php, avs[php])

        for hp in range(HP):
            av = [[avp.tile([VW, NSL], f32, tag="av", name=f"av{hp}_{hb}_{i}")
                   for i in range(IS)] for hb in range(2)]
            avs[hp] = av
            # interleaved projection work for the NEXT head pair, borrowing
            # scores-pool psum slots: (emit_at_j, which, slab). Each event
            # emits its two 512-col groups as two separate borrow tiles
            # (two slot turns back-to-back) so the sc rotation parity is
            # preserved and no single tensor burst exceeds the ACT lead.
            proj_work = {3: ("k", 0), 8: ("k", 2), 12: ("q", 0)} if hp + 1 < HP else {}
            wc_k = None

            for j in range(JT):
                if j in proj_work:
                    kind, sl0 = proj_work[j]
                    if kind == "k":
                        if sl0 == 0:
                            wc_k = load_wcol(wk, hp + 1, "wkc")
                        for g in range(2):
                            sl = sl0 + g
                            borrow = scp.tile([P, NSL], f32, tag="sc",
                                              name=f"bw{hp}_{j}_{g}")
                            kq_group(borrow, wc_k, ht_sb, sl)
                            nc.vector.tensor_scalar_mul(
                                kt_sb[hp + 1][:, sl * NSL:(sl + 1) * NSL],
                                borrow, W_INV)
                    else:
                        wc_q = load_wcol(wq, hp + 1, "wqc")
                        for g in range(IS):
                            borrow = scp.tile([P, NSL], f32, tag="sc",
                                              name=f"bw{hp}_{j}_{g}")
                            kq_group(borrow, wc_q, ht_sb, g)
                            nc.vector.tensor_scalar_mul(
                                qt_sb[hp + 1][:, g * NSL:(g + 1) * NSL],
                                borrow, W_INV)

                # one sc tile per query slab holding BOTH heads side by side
                # ([128 keys, h0-slab | h64-slab]): the pair of score matmuls
                # shares one rotation slot, so as soon as the slot frees both
                # can stream concurrently on disjoint PE row groups.
                # pt tiles pair consecutive key tiles ([128, 2, 1024] e5m2)
                # for the DoubleRow PV; exp(j) writes plane j%2. All pt
                # carry a uniform e^-2 factor (ACT bias / DVE mask constant)
                # for e5m2 overflow headroom - it cancels in the softmax.
                if j % 2 == 0:
                    cur_pair = [ptp.tile([P, 2, QLEN], f8e5, tag="pt",
                                         name=f"pt{hp}_{j}_{i}")
                                for i in range(IS)]
                for i in range(IS):
                    sc = scp.tile([P, QLEN], f32, tag="sc",
                                  name=f"sc{hp}_{j}_{i}")
                    for hb in range(2):
                        base = hb * 64
                        nc.tensor.matmul(
                            sc[:, hb * NSL:(hb + 1) * NSL],
                            kt_sb[hp][base:base + 64, j * P:(j + 1) * P],
                            qt_sb[hp][base:base + 64, i * NSL:(i + 1) * NSL],
                            start=True, stop=True, tile_position=(base, 0),
                            skip_group_check=(hb > 0))
                    pt_t = cur_pair[i][:, j % 2, :]
                    # route slab-1's exp to the DVE on odd key tiles:
                    # relieves the ACT engine and gives the scores-psum
                    # rotation a second, independent drain engine.
                    if i == 1 and j % 2 == 1:
                        scr = exs.tile([P, QLEN], f32, tag="ex",
                                       name=f"ex{hp}_{j}")
                        nc.vector._custom_dve(
                            EXP32_POLY, out=scr, in0=sc,
                            s0=SCALE / 32.0, s1=EXP_A, imm2=EXP_B)
                        nc.vector._custom_dve(
                            SQ32_MASK, out=pt_t, in0=scr,
                            s0=mmul_sb[:, j:j + 1])
                    else:
                        nc.scalar.activation(pt_t, sc, AF.Exp,
                                             bias=mask_sb[:, j:j + 1],
                                             scale=SCALE)

                if j % 2 == 1:
                    pv_pending.append((hp, j // 2, cur_pair))
                    flush_pv(1)

        flush_pv(0)

        # ---- output projection + residual + layernorm -----------------------
        attn_ctx.close()
        ph1_ctx.close()

        pso = ctx.enter_context(tc.tile_pool(name="pso", bufs=8, space="PSUM"))
        lnp = ctx.enter_context(tc.tile_pool(name="lnp", bufs=4))
        lns = ctx.enter_context(tc.tile_pool(name="lns", bufs=16))
        hqp = ctx.enter_context(tc.tile_pool(name="hqp", bufs=1))

        # prefetch the whole residual up front on both HWDGE queues so the
        # t-loop is compute-bound instead of waiting ~4.6 us of DMA per tile
        hq_tiles = []
        for t in range(TQ):
            hq_t = hqp.tile([P, D_MODEL], f32, name=f"hq{t}")
            eng = nc.sync if t % 2 == 0 else nc.scalar
            eng.dma_start(out=hq_t, in_=hq[t * P:(t + 1) * P, :])
            hq_tiles.append(hq_t)

        for t in range(TQ):
            hq_t = hq_tiles[t]
            xs = lnp.tile([P, D_MODEL], f32, tag="xs", name=f"xs{t}")
            sums = lns.tile([P, 2], f32, tag="sm", name=f"sm{t}")
            for m in range(2):
                ps = pso.tile([P, NSL], f32, tag="po", name=f"po{t}_{m}")
                for ep in range(ET // 2):
                    nc.tensor.matmul(
                        ps, avt_sb[ep][:, :, t * P:(t + 1) * P],
                        wo_sb[ep][:, :, m * NSL:(m + 1) * NSL],
                        start=(ep == 0), stop=(ep == ET // 2 - 1),
                        perf_mode=DR)
                # 1/256 undoes the x8 avt and x32 wo fp8 scales
                nc.vector.scalar_tensor_tensor(
                    out=xs[:, m * NSL:(m + 1) * NSL], in0=ps,
                    scalar=1.0 / 256.0,
                    in1=hq_t[:, m * NSL:(m + 1) * NSL],
                    op0=ALU.mult, op1=ALU.add,
                    accum_out=sums[:, m:m + 1])
            # mean/var via accum sums + ACT Square pass (keeps the tail off
            # the DVE): mean = (s0+s1)/D; var = sq/D - mean^2
            sq = lns.tile([P, 2], f32, tag="sq", name=f"sq{t}")
            xsq = lnp.tile([P, D_MODEL], f32, tag="xq", name=f"xq{t}")
            for m in range(2):
                nc.scalar.activation(xsq[:, m * NSL:(m + 1) * NSL],
                                     xs[:, m * NSL:(m + 1) * NSL], AF.Square,
                                     accum_out=sq[:, m:m + 1])
            mean = lns.tile([P, 1], f32, tag="mn", name=f"mn{t}")
            nc.vector.tensor_add(mean, sums[:, 0:1], sums[:, 1:2])
            nc.vector.tensor_scalar_mul(mean, mean, 1.0 / D_MODEL)
            msq = lns.tile([P, 1], f32, tag="mq", name=f"mq{t}")
            nc.vector.tensor_mul(msq, mean, mean)
            var = lns.tile([P, 1], f32, tag="vr", name=f"vr{t}")
            nc.vector.tensor_add(var, sq[:, 0:1], sq[:, 1:2])
            nc.vector.scalar_tensor_tensor(
                out=var, in0=var, scalar=1.0 / D_MODEL, in1=msq,
                op0=ALU.mult, op1=ALU.subtract)
            std = lns.tile([P, 1], f32, tag="sd", name=f"sd{t}")
            nc.scalar.activation(std, var, AF.Sqrt, bias=eps_sb[:, 0:1])
            rstd = lns.tile([P, 1], f32, tag="rs", name=f"rs{t}")
            nc.vector.reciprocal(rstd, std)
            nmr = lns.tile([P, 1], f32, tag="nm", name=f"nm{t}")
            nc.vector.tensor_scalar_mul(nmr, mean, -1.0)
            gs = lnp.tile([P, D_MODEL], f32, tag="gs", name=f"gs{t}")
            nc.vector.tensor_scalar(out=gs, in0=gam_sb,
                                    scalar1=rstd[:, 0:1], scalar2=None,
                                    op0=ALU.mult)
            xg = lnp.tile([P, D_MODEL], f32, tag="xg", name=f"xg{t}")
            nc.vector.scalar_tensor_tensor(
                out=xg, in0=xs, scalar=nmr[:, 0:1], in1=gs,
                op0=ALU.add, op1=ALU.mult)
            xn = lnp.tile([P, D_MODEL], f32, tag="xn", name=f"xn{t}")
            # the slow gpsimd add (2.3us) is fine mid-phase (pipelined away)
            # but sits on the drain path for the final tiles - do those on
            # the DVE instead.
            if t >= TQ - 2:
                nc.vector.tensor_add(xn, xg, bet_sb)
            else:
                nc.gpsimd.tensor_add(xn, xg, bet_sb)
            # spread the 4 MB output across both HWDGE queues in halves so
            # the write-back pipeline keeps pace with the t-loop
            for m in range(2):
                eng = [nc.sync, nc.scalar][(2 * t + m) % 2]
                eng.dma_start(
                    out=out[t * P:(t + 1) * P, m * NSL:(m + 1) * NSL],
                    in_=xn[:, m * NSL:(m + 1) * NSL])

    nc.compile()
    return nc


def _get_nc():
    if "nc" not in _CACHE:
        _CACHE["nc"] = _build()
    return _CACHE["nc"]


def _make_in_maps(inputs):
    f8 = getattr(ml_dtypes, "float8_e4m3fn", None) or ml_dtypes.float8_e4m3
    h = np.asarray(inputs["h"], dtype=np.float32)
    mask = np.asarray(inputs["attn_mask"])
    Wq = np.asarray(inputs["Wq"], dtype=np.float32)
    Wkv = np.asarray(inputs["Wkv"], dtype=np.float32)
    Wo = np.asarray(inputs["Wo"], dtype=np.float32)
    gamma = np.asarray(inputs["gamma"], dtype=np.float32)
    beta = np.asarray(inputs["beta"], dtype=np.float32)

    # weights x32 into the e4m3 normal range; kernel divides by 32 at the
    # psum evacuation (and 256 for the x8-scaled fp8 attn vectors @ Wo)
    wq_b = np.ascontiguousarray((Wq * 32).astype(f8))
    wk_b = np.ascontiguousarray((Wkv[:, :D_MODEL] * 32).astype(f8))
    wv_b = np.ascontiguousarray((Wkv[:, D_MODEL:] * 32).astype(f8))
    wo_b = np.ascontiguousarray((Wo * 32).astype(f8))

    in_maps = []
    for c in range(8):
        b, half = divmod(c, 2)
        hb = h[:, b, :]
        hT_b = hb.T.astype(f8)
        own = slice(half * QLEN, (half + 1) * QLEN)
        other = slice((1 - half) * QLEN, (2 - half) * QLEN)
        # own query-half first: keys are in core-local order, so the Q
        # projection can read the first half of hT uniformly on every core.
        # The mask is reordered identically; attention is key-order-invariant.
        hT_r = np.ascontiguousarray(np.concatenate(
            [hT_b[:, own], hT_b[:, other]], axis=1))
        # unmasked keys carry a uniform e^-2 (bias -2 on the ACT path,
        # multiplier e^-2 on the DVE path) for fp8e5m2 overflow headroom;
        # it divides out of the softmax exactly.
        mb_full = np.where(mask[:, b], np.float32(-1e9), np.float32(-2.0))
        mm_full = np.where(mask[:, b], np.float32(0.0),
                           np.float32(np.exp(-2.0)))
        in_maps.append({
            "hT": hT_r,
            "hq": np.ascontiguousarray(hb[own, :]),
            "wq": wq_b, "wk": wk_b, "wv": wv_b, "wo": wo_b,
            "mb": np.ascontiguousarray(
                np.concatenate([mb_full[own], mb_full[other]])),
            "mm": np.ascontiguousarray(
                np.concatenate([mm_full[own], mm_full[other]])),
            "gam": gamma, "bet": beta,
        })
    return in_maps


def _run(in_maps, **kwargs):
    from concourse.bass_utils import run_bass_kernel_spmd
    return run_bass_kernel_spmd(_get_nc(), in_maps, core_ids=list(range(8)),
                                **kwargs)


def kernel(**inputs) -> np.ndarray:
    res = _run(_make_in_maps(inputs))
    out = np.empty((SEQ, BSZ, D_MODEL), dtype=np.float32)
    for c in range(8):
        b, half = divmod(c, 2)
        out[half * QLEN:(half + 1) * QLEN, :, :][:, b, :] = res.results[c]["out"]
    return out



# revision 55
# speedup vs baseline: 1.1282x; 1.1282x over previous
"""MultiHeadAttn (post-LN, key-padding mask) Trainium2 Bass kernel, 8 cores.

Problem: h [S=2048, B=4, D=1024] f32; 16 heads x 64; key-padding mask [S, B];
out = LayerNorm(h + MHA(h)).

Sharding: core c handles batch b = c//2 and query half qh = c%2 (1024 query
rows), with all 16 heads and the full 2048-key context for that batch.
KV projections are recomputed by both cores of a batch pair (no collectives).

Per-core device pipeline (all matmuls bf16, fp32 accumulation in PSUM):
  - K^T/Q^T proj: stationary Wk/Wq column tiles, moving h^T -> [e, t] layout.
  - V proj: stationary h^T tiles, moving Wv -> natural [t, e] layout, stored
    with an appended ones column per head (gives softmax denominators via PV).
  - Attention per head pair: scores^T [j,i] via row-paired matmuls (two heads
    in row strips 0-63 / 64-127 of the PE array), exp via ScalarE with the
    key-padding bias as a per-partition bias and 1/sqrt(dh) as the scale,
    PV with ones-augmented V (M=65; row 64 accumulates the denominator),
    then normalize via reciprocal + partition broadcast + multiply.
  - Output proj: stationary attn_vec^T tiles, moving Wo; residual add + LN
    (bn_stats/bn_aggr) fused on DVE/ScalarE.
Next head pair's K/Q projections are interleaved into the attention loop
(borrowing scores-pool PSUM slots) so the PE stays busy under the ACT-bound
softmax stream.
"""
import numpy as np
import ml_dtypes

# ---- custom DVE exp: e^y = (p(y/32))^32 --------------------------------
# op1 (5 ALU stages): m = y*C0; p = (1+m) + m^2*(C1 + C2*m)  [f32]
# op2 (6 ALU stages): out = (in^32) * C0, C0 = per-partition 0/1 key mask
# Offloads part of the softmax exp stream from the ACT engine (the
# attention-loop bottleneck) to the otherwise idle DVE ALU pipeline.
# Fitted constants give <=0.55% rel err over |y|<=12 (f32), ~0.93% incl
# the bf16 store - the key-padding mask is exact (multiply by 0).
EXP_A = 0.5045829621045046
EXP_B = 0.16659614969050196


def _register_dve_exp():
    import concourse.dve_ops as dve_ops
    from concourse.dve_ops import DveOp
    from concourse.dve_spec import Spec, Src0, C0, C1, C2, One, sq, lower, _has_src1
    from concourse.dve_uop import DveOpSpec

    def reg(name, spec):
        if name in dve_ops._SUB_OPCODE_FOR_NAME:
            return next(o for o in dve_ops.OPS if o.name == name)
        row = max(dve_ops._SUB_OPCODE_FOR_NAME.values()) + 1
        shas = {}
        for ver in ("v3", "v4"):
            s = DveOpSpec(name=name, opcode=row, uops=lower(spec, ver=ver),
                          rd1_en=_has_src1(spec))
            shas[ver] = s.sha(ver)
        op = DveOp(name, spec, subdim=False, uops_sha=shas)
        dve_ops.OPS.append(op)
        dve_ops._SUB_OPCODE_FOR_NAME[name] = row
        dve_ops.CUSTOM_DVE_SPECS[name] = spec
        return op

    _m = Src0 * C0
    exp_poly = Spec(
        body=(One + _m) + sq(_m) * (C1 + C2 * _m),
        reference=lambda in0, in1, s0, s1, imm2: (
            (1.0 + in0 * s0) + (in0 * s0) ** 2 * (s1 + imm2 * (in0 * s0))
        ).astype(np.float32),
    )
    sq32_mask = Spec(
        body=sq(sq(sq(sq(sq(Src0))))) * C0,
        reference=lambda in0, in1, s0, s1, imm2: (
            (in0.astype(np.float64) ** 32) * s0
        ).astype(np.float32),
    )
    return reg("ANT_EXP32_POLY", exp_poly), reg("ANT_SQ32_MASK", sq32_mask)


N_HEAD, D_MODEL, D_HEAD = 16, 1024, 64
SEQ, BSZ = 2048, 4
QLEN = SEQ // 2
SCALE = 1.0 / D_HEAD ** 0.5
LN_EPS = 1e-5
P = 128
NSL = 512                   # matmul moving-operand slab (one PSUM bank fp32)
CT = D_MODEL // P           # 8 contraction tiles
ET = D_MODEL // P           # 8 e-tiles (2 heads each)
JT = SEQ // P               # 16 key tiles
JS = SEQ // NSL             # 4 key slabs
IS = QLEN // NSL            # 2 query slabs
TQ = QLEN // P              # 8 query-row tiles
HP = N_HEAD // 2            # 8 head pairs

_CACHE = {}


def _build():
    from contextlib import ExitStack
    import concourse.bass as bass
    import concourse.mybir as mybir
    import concourse.tile as tile
    from concourse import bacc

    dt = mybir.dt
    f32, bf16 = dt.float32, dt.bfloat16
    AF = mybir.ActivationFunctionType
    ALU = mybir.AluOpType
    EXP32_POLY, SQ32_MASK = _register_dve_exp()

    nc = bacc.Bacc(None, target_bir_lowering=False)

    f8 = dt.float8e4
    # all four projections run fp8e4m3 DoubleRow (two 128-contraction tiles
    # per PE pass): h is quantized as-is (|h|<~6), weights are pre-scaled
    # x32 on the host so their ~N(0,1/32) entries use the e4m3 normal range;
    # the 1/32 rides existing evacuation multiplies for free.
    hT = nc.dram_tensor("hT", [D_MODEL, SEQ], f8, kind="ExternalInput")
    hq = nc.dram_tensor("hq", [QLEN, D_MODEL], f32, kind="ExternalInput")
    wq = nc.dram_tensor("wq", [D_MODEL, D_MODEL], f8, kind="ExternalInput")
    wk = nc.dram_tensor("wk", [D_MODEL, D_MODEL], f8, kind="ExternalInput")
    wv = nc.dram_tensor("wv", [D_MODEL, D_MODEL], f8, kind="ExternalInput")
    wo = nc.dram_tensor("wo", [D_MODEL, D_MODEL], f8, kind="ExternalInput")
    mb = nc.dram_tensor("mb", [SEQ], f32, kind="ExternalInput")
    mm = nc.dram_tensor("mm", [SEQ], f32, kind="ExternalInput")
    gam = nc.dram_tensor("gam", [D_MODEL], f32, kind="ExternalInput")
    bet = nc.dram_tensor("bet", [D_MODEL], f32, kind="ExternalInput")
    out = nc.dram_tensor("out", [QLEN, D_MODEL], f32, kind="ExternalOutput")

    with tile.TileContext(nc) as tc, ExitStack() as ctx:
        persist = ctx.enter_context(tc.tile_pool(name="persist", bufs=1))

        # normalized attn vectors stored fp8 (x8 for e4m3 range), paired
        # along the contraction dim for the DoubleRow output projection
        avt_sb = [persist.tile([P, 2, QLEN], f8, name=f"avt{ep}")
                  for ep in range(ET // 2)]
        mask_sb = persist.tile([P, JT], f32, name="mask")
        mmul_sb = persist.tile([P, JT], f32, name="mmul")
        eps_sb = persist.tile([P, 1], f32, name="eps")

        nc.vector.memset(eps_sb, LN_EPS)

        nc.gpsimd.dma_start(out=mask_sb,
                            in_=bass.AP(tensor=mb, offset=0, ap=[[1, P], [P, JT]]))
        nc.gpsimd.dma_start(out=mmul_sb,
                            in_=bass.AP(tensor=mm, offset=0, ap=[[1, P], [P, JT]]))

        # ---- phase-3 weights: load early into the region wvp freed ---------
        w3p = ctx.enter_context(tc.tile_pool(name="w3p", bufs=1))
        wo_sb = [w3p.tile([P, 2, D_MODEL], f8, name=f"wo{cp}")
                 for cp in range(CT // 2)]
        gam_sb = w3p.tile([P, D_MODEL], f32, name="gamr")
        bet_sb = w3p.tile([P, D_MODEL], f32, name="betr")

        # ---- phase 1 scope: h^T residency + streamed W columns --------------
        # h^T lives as 4 contraction-pair tiles [128, 2, SEQ] fp8 so a
        # DoubleRow matmul consumes two 128-row tiles per pass.
        ph1_ctx = ExitStack()
        ph1 = ph1_ctx.enter_context(tc.tile_pool(name="ph1", bufs=1))
        ht_sb = [ph1.tile([P, 2, SEQ], f8, name=f"ht{cp}")
                 for cp in range(CT // 2)]
        # attention-lifetime tensors live in this scope (not persist) so the
        # post phase gets their SBUF back for deeper LN pipelining
        kt_sb = [ph1.tile([P, SEQ], bf16, name=f"kt{e}") for e in range(ET)]
        qt_sb = [ph1.tile([P, QLEN], bf16, name=f"qt{e}") for e in range(ET)]
        v_sb = [ph1.tile([P, N_HEAD, D_HEAD], bf16, name=f"v{t}")
                for t in range(JT)]
        ones64 = ph1.tile([P, 64], bf16, name="ones64")
        nc.vector.memset(ones64, 1.0)

        wcol = ph1_ctx.enter_context(tc.tile_pool(name="wcol", bufs=3))

        def load_wcol(w, e, tag):
            wc = wcol.tile([P, CT, P], f8, tag=tag, name=f"{tag}{e}")
            nc.sync.dma_start(
                out=wc,
                in_=w[:, e * P:(e + 1) * P].rearrange("(ct p) e -> p ct e", p=P))
            return wc

        DR = mybir.MatmulPerfMode.DoubleRow

        wc0 = load_wcol(wk, 0, "wkc")
        # stream h^T by key slab so the first K-proj group (slab 0) starts
        # after ~0.5 MB of DMA instead of the full 2 MB.
        for sl in range(JS):
            for c in range(CT):
                eng = nc.sync if (sl * CT + c) % 2 == 0 else nc.scalar
                eng.dma_start(
                    out=ht_sb[c // 2][:, c % 2, sl * NSL:(sl + 1) * NSL],
                    in_=hT[c * P:(c + 1) * P, sl * NSL:(sl + 1) * NSL])

        def kq_group(ps_ap, wc, moving, sl):
            """4 DoubleRow matmuls: one K/Q-proj output group into psum."""
            for cp in range(CT // 2):
                nc.tensor.matmul(ps_ap, wc[:, 2 * cp:2 * cp + 2, :],
                                 moving[cp][:, :, sl * NSL:(sl + 1) * NSL],
                                 start=(cp == 0), stop=(cp == CT // 2 - 1),
                                 perf_mode=DR)

        # prephase: K(0), Q(0), V (own pools, closed before attention).
        # Evacuations fold the 1/32 fp8 weight-scale compensation into the
        # psum->bf16 cast.
        W_INV = 1.0 / 32.0
        with tc.tile_pool(name="wvp", bufs=1) as wvp, \
             tc.tile_pool(name="psA", bufs=6, space="PSUM") as psA:
            wv_sb = [wvp.tile([P, 2, D_MODEL], f8, name=f"wv{cp}")
                     for cp in range(CT // 2)]
            for c in range(CT):
                nc.scalar.dma_start(out=wv_sb[c // 2][:, c % 2, :],
                                    in_=wv[c * P:(c + 1) * P, :])
            wc = wc0
            for j in range(JS):
                ps = psA.tile([P, NSL], f32, tag="psa", name=f"psk0_{j}")
                kq_group(ps, wc, ht_sb, j)
                nc.vector.tensor_scalar_mul(
                    kt_sb[0][:, j * NSL:(j + 1) * NSL], ps, W_INV)
            wc = load_wcol(wq, 0, "wqc")
            for i in range(IS):
                ps = psA.tile([P, NSL], f32, tag="psa", name=f"psq0_{i}")
                kq_group(ps, wc, ht_sb, i)
                nc.vector.tensor_scalar_mul(
                    qt_sb[0][:, i * NSL:(i + 1) * NSL], ps, W_INV)
            # V projection: stationary h^T pair tiles, moving Wv slabs
            for t in range(JT):
                for es in range(2):
                    ps = psA.tile([P, NSL], f32, tag="psa", name=f"psv{t}_{es}")
                    for cp in range(CT // 2):
                        nc.tensor.matmul(
                            ps, ht_sb[cp][:, :, t * P:(t + 1) * P],
                            wv_sb[cp][:, :, es * NSL:(es + 1) * NSL],
                            start=(cp == 0), stop=(cp == CT // 2 - 1),
                            perf_mode=DR)
                    nc.vector.tensor_scalar_mul(
                        v_sb[t][:, es * 8:(es + 1) * 8, :],
                        ps[:, :].rearrange("p (h d) -> p h d", d=D_HEAD),
                        W_INV)

        def emit_pv(nc, v_sb, ones64, av, den, hp, j, pts):
            # pts[i] is [128 keys, 1024] = [head-A slab-i | head-B slab-i]
            first, last = (j == 0), (j == JT - 1)
            ha = slice(0, NSL)
            hb_ = slice(NSL, 2 * NSL)
            for i in range(IS):
                nc.tensor.matmul(av[i][0:64, :], v_sb[j][:, hp * 2, :],
                                 pts[i][:, ha], start=first, stop=last,
                                 tile_position=(0, 0))
                nc.tensor.matmul(av[i][64:P, :], v_sb[j][:, hp * 2 + 1, :],
                                 pts[i][:, hb_], start=first, stop=last,
                                 tile_position=(0, 64), skip_group_check=True)
                nc.tensor.matmul(den[i][0:64, :], ones64, pts[i][:, ha],
                                 start=first, stop=last,
                                 tile_position=(0, 0), skip_group_check=True)
                nc.tensor.matmul(den[i][64:P, :], ones64, pts[i][:, hb_],
                                 start=first, stop=last,
                                 tile_position=(0, 64), skip_group_check=True)

        for c in range(CT):
            nc.scalar.dma_start(out=wo_sb[c // 2][:, c % 2, :],
                                in_=wo[c * P:(c + 1) * P, :])
        nc.gpsimd.dma_start(out=gam_sb,
                            in_=bass.AP(tensor=gam, offset=0, ap=[[0, P], [1, D_MODEL]]))
        nc.gpsimd.dma_start(out=bet_sb,
                            in_=bass.AP(tensor=bet, offset=0, ap=[[0, P], [1, D_MODEL]]))

        # ---- attention ------------------------------------------------------
        attn_ctx = ExitStack()
        scp = attn_ctx.enter_context(tc.tile_pool(name="scp", bufs=2, space="PSUM"))
        avp = attn_ctx.enter_context(tc.tile_pool(name="avp", bufs=2, space="PSUM"))
        ptp = attn_ctx.enter_context(tc.tile_pool(name="ptp", bufs=8))
        nrm = attn_ctx.enter_context(tc.tile_pool(name="nrm", bufs=3))
        exs = attn_ctx.enter_context(tc.tile_pool(name="exs", bufs=2))

        def emit_norm(hp, av, den):
            # normalize straight out of PSUM: fast-approx reciprocal of the
            # replicated denominators, then one STT per head fusing the
            # x8 fp8-range scale and the multiply with the fp8 store.
            for i in range(IS):
                rep = nrm.tile([P, NSL], f32, tag="rep", name=f"rep{hp}_{i}")
                nc.vector.reciprocal_approx_fast(rep, den[i])
                for hb in range(2):
                    nc.vector.scalar_tensor_tensor(
                        out=avt_sb[hp // 2][hb * 64:(hb + 1) * 64, hp % 2,
                                            i * NSL:(i + 1) * NSL],
                        in0=av[i][hb * 64:(hb + 1) * 64, :], scalar=8.0,
                        in1=rep[hb * 64:(hb + 1) * 64, :],
                        op0=ALU.mult, op1=ALU.mult)

        # PV (and the hp-final normalization) trail the scores/exp stream by
        # two key tiles GLOBALLY - the pipeline flows across head-pair
        # boundaries, so the last exps of one hp overlap the first scores
        # of the next instead of draining into a bubble.
        avs = {}
        pv_pending = []

        def flush_pv(upto):
            while len(pv_pending) > upto:
                php, pj, ppts = pv_pending.pop(0)
                pav, pden = avs[php]
                emit_pv(nc, v_sb, ones64, pav, pden, php, pj, ppts)
                if pj == JT - 1:
                    emit_norm(php, pav, pden)

        for hp in range(HP):
            av = [avp.tile([P, NSL], f32, tag="av", name=f"av{hp}_{i}")
                  for i in range(IS)]
            den = [avp.tile([P, NSL], f32, tag="den", name=f"den{hp}_{i}")
                   for i in range(IS)]
            avs[hp] = (av, den)
            # interleaved projection work for the NEXT head pair, borrowing
            # scores-pool psum slots: (emit_at_j, which, slab). Each event
            # emits its two 512-col groups as two separate borrow tiles
            # (two slot turns back-to-back) so the sc rotation parity is
            # preserved and no single tensor burst exceeds the ACT lead.
            proj_work = {3: ("k", 0), 8: ("k", 2), 12: ("q", 0)} if hp + 1 < HP else {}
            wc_k = None

            for j in range(JT):
                if j in proj_work:
                    kind, sl0 = proj_work[j]
                    if kind == "k":
                        if sl0 == 0:
                            wc_k = load_wcol(wk, hp + 1, "wkc")
                        for g in range(2):
                            sl = sl0 + g
                            borrow = scp.tile([P, NSL], f32, tag="sc",
                                              name=f"bw{hp}_{j}_{g}")
                            kq_group(borrow, wc_k, ht_sb, sl)
                            nc.vector.tensor_scalar_mul(
                                kt_sb[hp + 1][:, sl * NSL:(sl + 1) * NSL],
                                borrow, W_INV)
                    else:
                        wc_q = load_wcol(wq, hp + 1, "wqc")
                        for g in range(IS):
                            borrow = scp.tile([P, NSL], f32, tag="sc",
                                              name=f"bw{hp}_{j}_{g}")
                            kq_group(borrow, wc_q, ht_sb, g)
                            nc.vector.tensor_scalar_mul(
                                qt_sb[hp + 1][:, g * NSL:(g + 1) * NSL],
                                borrow, W_INV)

                cur_pt = []
                # one sc tile per query slab holding BOTH heads side by side
                # ([128 keys, h0-slab | h64-slab]): the pair of score matmuls
                # shares one rotation slot, so as soon as the slot frees both
                # can stream concurrently on disjoint PE row groups; per-head
                # tiles sat in different slots that freed ~1us apart, which
                # serialized every pair.
                for i in range(IS):
                    sc = scp.tile([P, QLEN], f32, tag="sc",
                                  name=f"sc{hp}_{j}_{i}")
                    for hb in range(2):
                        base = hb * 64
                        nc.tensor.matmul(
                            sc[:, hb * NSL:(hb + 1) * NSL],
                            kt_sb[hp][base:base + 64, j * P:(j + 1) * P],
                            qt_sb[hp][base:base + 64, i * NSL:(i + 1) * NSL],
                            start=True, stop=True, tile_position=(base, 0),
                            skip_group_check=(hb > 0))
                    pt_t = ptp.tile([P, QLEN], bf16, tag="pt",
                                    name=f"pt{hp}_{j}_{i}")
                    # route slab-1's exp to the DVE on selected key tiles:
                    # relieves the ACT engine and gives the scores-psum
                    # rotation a second, independent drain engine. Tiles
                    # right before a proj event stay on ACT so the event's
                    # DVE casts aren't queued behind a 2.5us exp chain.
                    if i == 1 and j % 2 == 1:
                        scr = exs.tile([P, QLEN], f32, tag="ex",
                                       name=f"ex{hp}_{j}")
                        nc.vector._custom_dve(
                            EXP32_POLY, out=scr, in0=sc,
                            s0=SCALE / 32.0, s1=EXP_A, imm2=EXP_B)
                        nc.vector._custom_dve(
                            SQ32_MASK, out=pt_t, in0=scr,
                            s0=mmul_sb[:, j:j + 1])
                    else:
                        nc.scalar.activation(pt_t, sc, AF.Exp,
                                             bias=mask_sb[:, j:j + 1],
                                             scale=SCALE)
                    cur_pt.append(pt_t)

                pv_pending.append((hp, j, cur_pt))
                flush_pv(2)

        flush_pv(0)

        # ---- output projection + residual + layernorm -----------------------
        attn_ctx.close()
        ph1_ctx.close()

        pso = ctx.enter_context(tc.tile_pool(name="pso", bufs=8, space="PSUM"))
        lnp = ctx.enter_context(tc.tile_pool(name="lnp", bufs=3))
        lns = ctx.enter_context(tc.tile_pool(name="lns", bufs=16))
        hqp = ctx.enter_context(tc.tile_pool(name="hqp", bufs=1))

        # prefetch the whole residual up front on both HWDGE queues so the
        # t-loop is compute-bound instead of waiting ~4.6 us of DMA per tile
        hq_tiles = []
        for t in range(TQ):
            hq_t = hqp.tile([P, D_MODEL], f32, name=f"hq{t}")
            eng = nc.sync if t % 2 == 0 else nc.scalar
            eng.dma_start(out=hq_t, in_=hq[t * P:(t + 1) * P, :])
            hq_tiles.append(hq_t)

        for t in range(TQ):
            hq_t = hq_tiles[t]
            xs = lnp.tile([P, D_MODEL], f32, tag="xs", name=f"xs{t}")
            sums = lns.tile([P, 2], f32, tag="sm", name=f"sm{t}")
            for m in range(2):
                ps = pso.tile([P, NSL], f32, tag="po", name=f"po{t}_{m}")
                for ep in range(ET // 2):
                    nc.tensor.matmul(
                        ps, avt_sb[ep][:, :, t * P:(t + 1) * P],
                        wo_sb[ep][:, :, m * NSL:(m + 1) * NSL],
                        start=(ep == 0), stop=(ep == ET // 2 - 1),
                        perf_mode=DR)
                # 1/256 undoes the x8 avt and x32 wo fp8 scales
                nc.vector.scalar_tensor_tensor(
                    out=xs[:, m * NSL:(m + 1) * NSL], in0=ps,
                    scalar=1.0 / 256.0,
                    in1=hq_t[:, m * NSL:(m + 1) * NSL],
                    op0=ALU.mult, op1=ALU.add,
                    accum_out=sums[:, m:m + 1])
            # mean/var via accum sums + ACT Square pass (keeps the tail off
            # the DVE): mean = (s0+s1)/D; var = sq/D - mean^2
            sq = lns.tile([P, 2], f32, tag="sq", name=f"sq{t}")
            xsq = lnp.tile([P, D_MODEL], f32, tag="xq", name=f"xq{t}")
            for m in range(2):
                nc.scalar.activation(xsq[:, m * NSL:(m + 1) * NSL],
                                     xs[:, m * NSL:(m + 1) * NSL], AF.Square,
                                     accum_out=sq[:, m:m + 1])
            mean = lns.tile([P, 1], f32, tag="mn", name=f"mn{t}")
            nc.vector.tensor_add(mean, sums[:, 0:1], sums[:, 1:2])
            nc.vector.tensor_scalar_mul(mean, mean, 1.0 / D_MODEL)
            msq = lns.tile([P, 1], f32, tag="mq", name=f"mq{t}")
            nc.vector.tensor_mul(msq, mean, mean)
            var = lns.tile([P, 1], f32, tag="vr", name=f"vr{t}")
            nc.vector.tensor_add(var, sq[:, 0:1], sq[:, 1:2])
            nc.vector.scalar_tensor_tensor(
                out=var, in0=var, scalar=1.0 / D_MODEL, in1=msq,
                op0=ALU.mult, op1=ALU.subtract)
            std = lns.tile([P, 1], f32, tag="sd", name=f"sd{t}")
            nc.scalar.activation(std, var, AF.Sqrt, bias=eps_sb[:, 0:1])
            rstd = lns.tile([P, 1], f32, tag="rs", name=f"rs{t}")
            nc.vector.reciprocal(rstd, std)
            nmr = lns.tile([P, 1], f32, tag="nm", name=f"nm{t}")
            nc.vector.tensor_scalar_mul(nmr, mean, -1.0)
            gs = lnp.tile([P, D_MODEL], f32, tag="gs", name=f"gs{t}")
            nc.vector.tensor_scalar(out=gs, in0=gam_sb,
                                    scalar1=rstd[:, 0:1], scalar2=None,
                                    op0=ALU.mult)
            xg = lnp.tile([P, D_MODEL], f32, tag="xg", name=f"xg{t}")
            nc.vector.scalar_tensor_tensor(
                out=xg, in0=xs, scalar=nmr[:, 0:1], in1=gs,
                op0=ALU.add, op1=ALU.mult)
            xn = lnp.tile([P, D_MODEL], f32, tag="xn", name=f"xn{t}")
            # the slow gpsimd add (2.3us) is fine mid-phase (pipelined away)
            # but sits on the drain path for the final tiles - do those on
            # the DVE instead.
            if t >= TQ - 2:
                nc.vector.tensor_add(xn, xg, bet_sb)
            else:
                nc.gpsimd.tensor_add(xn, xg, bet_sb)
            # spread the 4 MB output across both HWDGE queues in halves so
            # the write-back pipeline keeps pace with the t-loop
            for m in range(2):
                eng = [nc.sync, nc.scalar][(2 * t + m) % 2]
                eng.dma_start(
                    out=out[t * P:(t + 1) * P, m * NSL:(m + 1) * NSL],
                    in_=xn[:, m * NSL:(m + 1) * NSL])

    nc.compile()
    return nc


def _get_nc():
    if "nc" not in _CACHE:
        _CACHE["nc"] = _build()
    return _CACHE["nc"]


def _make_in_maps(inputs):
    f8 = getattr(ml_dtypes, "float8_e4m3fn", None) or ml_dtypes.float8_e4m3
    h = np.asarray(inputs["h"], dtype=np.float32)
    mask = np.asarray(inputs["attn_mask"])
    Wq = np.asarray(inputs["Wq"], dtype=np.float32)
    Wkv = np.asarray(inputs["Wkv"], dtype=np.float32)
    Wo = np.asarray(inputs["Wo"], dtype=np.float32)
    gamma = np.asarray(inputs["gamma"], dtype=np.float32)
    beta = np.asarray(inputs["beta"], dtype=np.float32)

    # weights x32 into the e4m3 normal range; kernel divides by 32 at the
    # psum evacuation (and 256 for the x8-scaled fp8 attn vectors @ Wo)
    wq_b = np.ascontiguousarray((Wq * 32).astype(f8))
    wk_b = np.ascontiguousarray((Wkv[:, :D_MODEL] * 32).astype(f8))
    wv_b = np.ascontiguousarray((Wkv[:, D_MODEL:] * 32).astype(f8))
    wo_b = np.ascontiguousarray((Wo * 32).astype(f8))

    in_maps = []
    for c in range(8):
        b, half = divmod(c, 2)
        hb = h[:, b, :]
        hT_b = hb.T.astype(f8)
        own = slice(half * QLEN, (half + 1) * QLEN)
        other = slice((1 - half) * QLEN, (2 - half) * QLEN)
        # own query-half first: keys are in core-local order, so the Q
        # projection can read the first half of hT uniformly on every core.
        # The mask is reordered identically; attention is key-order-invariant.
        hT_r = np.ascontiguousarray(np.concatenate(
            [hT_b[:, own], hT_b[:, other]], axis=1))
        mb_full = np.where(mask[:, b], np.float32(-1e9), np.float32(0.0))
        mm_full = np.where(mask[:, b], np.float32(0.0), np.float32(1.0))
        in_maps.append({
            "hT": hT_r,
            "hq": np.ascontiguousarray(hb[own, :]),
            "wq": wq_b, "wk": wk_b, "wv": wv_b, "wo": wo_b,
            "mb": np.ascontiguousarray(
                np.concatenate([mb_full[own], mb_full[other]])),
            "mm": np.ascontiguousarray(
                np.concatenate([mm_full[own], mm_full[other]])),
            "gam": gamma, "bet": beta,
        })
    return in_maps


def _run(in_maps, **kwargs):
    from concourse.bass_utils import run_bass_kernel_spmd
    return run_bass_kernel_spmd(_get_nc(), in_maps, core_ids=list(range(8)),
                                **kwargs)


def kernel(**inputs) -> np.ndarray:
    res = _run(_make_in_maps(inputs))
    out = np.empty((SEQ, BSZ, D_MODEL), dtype=np.float32)
    for c in range(8):
        b, half = divmod(c, 2)
        out[half * QLEN:(half + 1) * QLEN, :, :][:, b, :] = res.results[c]["out"]
    return out

